# revision 5
# baseline (speedup 1.0000x reference)
"""Trainium2 Bass kernel for the CrossEntropyMap loss.

Math (per batch row b of y_hat[B=64, T=64, G=128, G]):
    lse_b  = logsumexp(y_hat[b].reshape(-1))            # over T*G*G = 1M classes
    pick_b = sum_t y_hat[b, t, xi[b,t], yi[b,t]]        # xi/yi = round(coords*G)
    loss   = mean_b(T * lse_b - pick_b)

Sharding: data-parallel over batch, 8 rows per NeuronCore (32 MiB/core).
The device's only job is the bandwidth-bound part: stream the 32 MiB shard
from HBM and accumulate per-partition sums of exp(x + C_SHIFT) on the ACT
engine (accum_out). Any constant shift is mathematically exact for logsumexp
(it only scales the partial sums); C_SHIFT=-16 keeps exp in range for |x| up
to ~100.

Everything else runs on the host in float64: the 4096 picked logits are pure
indexing into y_hat (already resident in host memory), and the final
cross-partition sums + ln + mean are O(B) work. Shipping only the raw
[128, 11] accumulator tile per core keeps the device critical path to
"last chunk load -> one short exp -> one 5.5 KB store" and minimizes the
instruction/semaphore count (the profiled end-of-kernel semaphore teardown
scales with it; the previous on-device reduction paid ~9 us there).

Chunking: rows 0..6 stream as full-row [128, 8192] transfers (32 KB
descriptors halve the DMA descriptor count vs half-rows); row 7 streams as
four [128, 2048] quarters so the final exp on the critical path is ~0.9 us
instead of 7.4. Exp runs in place over the input tile, so no scratch pool.
The DMA completion tail is set by the slowest of the 16 DMA engines (each
[128, F] transfer stripes one descriptor per partition round-robin over all
16), so chunk sizing does not change the load time itself.

Engine placement: prefilled transfers alternate between the SP and ACT
HWDGE rings (both otherwise idle); steady-state transfers all go to the SP
ring, which stays ahead since ACT consumes slower than the engines load.
"""

import sys

import numpy as np

try:
    import concourse.bacc as bacc
except ImportError:  # pragma: no cover - fallback for bare environments
    sys.path.insert(0, "/opt/trn_rl_repo")
    import concourse.bacc as bacc

import concourse.tile as tile
from concourse import mybir
from concourse.bass_utils import run_bass_kernel_spmd

B, T, G = 64, 64, 128
N_CORES = 8
ROWS = B // N_CORES            # 8 batch rows per core
ROW_ELEMS = T * G * G          # 1_048_576 classes per row
P = 128
F = ROW_ELEMS // P             # 8192 elements per partition per row
N_PER_CORE = ROWS * ROW_ELEMS  # 8_388_608 elements per core shard
C_SHIFT = -16.0                # constant exp bias (exact for logsumexp)

N_BIG = ROWS - 1               # rows 0..6 as full-row transfers
SMALL_SPLIT = 4                # row 7 in quarters
FH_SMALL = F // SMALL_SPLIT    # 2048
N_CHUNKS = N_BIG + SMALL_SPLIT # 11 chunk columns in the output

_f32 = mybir.dt.float32
_EXP = mybir.ActivationFunctionType.Exp

_compiled_nc = None

# Test hook: BassKernelResults of the last run.
LAST_RESULTS = None


def build_nc():
    nc = bacc.Bacc("TRN2", target_bir_lowering=False, debug=False)
    y = nc.dram_tensor("y", [N_PER_CORE, 1], _f32, kind="ExternalInput")
    out = nc.dram_tensor("out", [P, N_CHUNKS], _f32, kind="ExternalOutput")

    # [ROWS, 128, 8192] row view: partition p of row r holds elements
    # [r*1M + p*8192, +8192) — contiguous per partition.
    y_rows = y.ap().rearrange("(r p f) o -> r p (f o)", r=ROWS, p=P)

    # chunk c -> (row, f_start, f_len); smalls last so the tail exp is short.
    chunks = [(r, 0, F) for r in range(N_BIG)] + [
        (ROWS - 1, q * FH_SMALL, FH_SMALL) for q in range(SMALL_SPLIT)
    ]

    with tile.TileContext(nc) as tc:
        with (
            tc.tile_pool(name="bigpool", bufs=4) as bigpool,
            tc.tile_pool(name="smallpool", bufs=SMALL_SPLIT) as smallpool,
            tc.tile_pool(name="small", bufs=1) as small,
        ):
            cbias = small.tile([P, 1], _f32)
            nc.vector.memset(cbias[:], C_SHIFT)

            # res[:, c] = per-partition sum of exp(chunk c + C_SHIFT).
            res = small.tile([P, N_CHUNKS], _f32)

            x_tiles = {}

            def issue_dma(c, eng):
                r, f0, fl = chunks[c]
                pool = bigpool if fl == F else smallpool
                xt = pool.tile([P, fl], _f32, tag="x")
                eng.dma_start(out=xt[:], in_=y_rows[r][:, f0 : f0 + fl])
                x_tiles[c] = xt

            prefill = 4
            for c in range(prefill):
                issue_dma(c, nc.sync if c % 2 == 0 else nc.scalar)
            for c in range(N_CHUNKS):
                xt = x_tiles.pop(c)
                nc.scalar.activation(
                    out=xt[:], in_=xt[:], func=_EXP, bias=cbias[:, 0:1],
                    scale=1.0, accum_out=res[:, c : c + 1],
                )
                if c + prefill < N_CHUNKS:
                    issue_dma(c + prefill, nc.sync)

            nc.sync.dma_start(out=out.ap(), in_=res[:])

    nc.compile()
    return nc


def make_in_maps(y_hat: np.ndarray):
    """Shard y_hat across cores (8 batch rows each)."""
    y_hat = np.ascontiguousarray(y_hat, dtype=np.float32)
    return [
        {"y": y_hat[c * ROWS : (c + 1) * ROWS].reshape(N_PER_CORE, 1)}
        for c in range(N_CORES)
    ]


def kernel(y_hat: np.ndarray, coords: np.ndarray) -> np.ndarray:
    global _compiled_nc, LAST_RESULTS
    y_hat = np.ascontiguousarray(y_hat, dtype=np.float32)
    coords = np.asarray(coords, dtype=np.float32)
    if _compiled_nc is None:
        _compiled_nc = build_nc()
    res = run_bass_kernel_spmd(
        _compiled_nc, make_in_maps(y_hat), core_ids=list(range(N_CORES))
    )
    LAST_RESULTS = res

    # Host-side finish in float64. Picked logits are pure indexing; matching
    # jnp.round (round-half-to-even) is np.round's semantics, and coords*128
    # is exact in f32 (power-of-two scale).
    xi = np.round(coords[:, :, 0] * np.float32(G)).astype(np.int64)  # (B, T)
    yi = np.round(coords[:, :, 1] * np.float32(G)).astype(np.int64)  # (B, T)
    bi = np.arange(B, dtype=np.int64)[:, None]
    ti = np.arange(T, dtype=np.int64)[None, :]
    picksum = y_hat[bi, ti, xi, yi].astype(np.float64).sum()

    lse_sum = 0.0
    for c, r in enumerate(res.results):
        o = np.asarray(r["out"]).astype(np.float64)      # [P, N_CHUNKS]
        s = o.sum(axis=0)                                # per-chunk totals
        s_rows = np.concatenate([s[:N_BIG], [s[N_BIG:].sum()]])
        lse_sum += (np.log(s_rows) - C_SHIFT).sum()
    return np.array(np.float32((T * lse_sum - picksum) / B))


# revision 11
# speedup vs baseline: 1.0402x; 1.0402x over previous
"""Trainium2 Bass kernel for the CrossEntropyMap loss.

Math (per batch row b of y_hat[B=64, T=64, G=128, G]):
    lse_b  = logsumexp(y_hat[b].reshape(-1))            # over T*G*G = 1M classes
    pick_b = sum_t y_hat[b, t, xi[b,t], yi[b,t]]        # xi/yi = round(coords*G)
    loss   = mean_b(T * lse_b - pick_b)

Sharding: data-parallel over batch, 8 rows per NeuronCore (32 MiB/core).
The device's only job is the bandwidth-bound part: stream the 32 MiB shard
from HBM and accumulate per-partition sums of exp(x + C_SHIFT) on the ACT
engine (accum_out). Any constant shift is mathematically exact for logsumexp
(it only scales the partial sums); C_SHIFT=-16 keeps exp in range for |x| up
to ~100.

Everything else runs on the host in float64: the 4096 picked logits are pure
indexing into y_hat (already resident in host memory), and the final
cross-partition sums + ln + mean are O(B) work. Shipping only the raw
[128, 11] accumulator tile per core keeps the device critical path to
"last chunk load -> one short exp -> one 5.5 KB store" and minimizes the
instruction/semaphore count (the profiled end-of-kernel semaphore teardown
scales with it; the previous on-device reduction paid ~9 us there).

Chunking: 16 uniform half-row [128, 4096] transfers, ALL on the SP HWDGE
ring in consumption order. A single queue delivers chunk completions in
exactly the order the exp stream needs them at the full ~425 GB/s the 16
DMA engines sustain; splitting across two rings makes each ring drain its
own backlog serially, so a ring's k-th transfer completes far later than
aggregate bandwidth suggests and stalls ACT mid-kernel (measured +10 us).
Exp runs in place over the input tile, so no scratch pool. Chunk-size
tapering buys nothing: the ~460 ns fixed cost per extra activation +
accumulator-readout cancels the shorter tail exp (simulated both).
"""

import sys

import numpy as np

try:
    import concourse.bacc as bacc
except ImportError:  # pragma: no cover - fallback for bare environments
    sys.path.insert(0, "/opt/trn_rl_repo")
    import concourse.bacc as bacc

import concourse.tile as tile
from concourse import mybir
from concourse.bass_utils import run_bass_kernel_spmd

B, T, G = 64, 64, 128
N_CORES = 8
ROWS = B // N_CORES            # 8 batch rows per core
ROW_ELEMS = T * G * G          # 1_048_576 classes per row
P = 128
F = ROW_ELEMS // P             # 8192 elements per partition per row
N_PER_CORE = ROWS * ROW_ELEMS  # 8_388_608 elements per core shard
C_SHIFT = -16.0                # constant exp bias (exact for logsumexp)

HALVES = 2                     # chunks per row
FH = F // HALVES               # 4096 cols per chunk (2 MiB)
N_CHUNKS = ROWS * HALVES       # 16 chunk columns in the output

_f32 = mybir.dt.float32
_EXP = mybir.ActivationFunctionType.Exp

_compiled_nc = None

# Test hook: BassKernelResults of the last run.
LAST_RESULTS = None


def build_nc():
    nc = bacc.Bacc("TRN2", target_bir_lowering=False, debug=False)
    y = nc.dram_tensor("y", [N_PER_CORE, 1], _f32, kind="ExternalInput")
    out = nc.dram_tensor("out", [P, N_CHUNKS], _f32, kind="ExternalOutput")

    # [ROWS, HALVES, 128, 4096] chunk view: partition p of chunk (r, h) holds
    # elements [r*1M + p*8192 + h*4096, +4096) — contiguous per partition.
    y_chunks = y.ap().rearrange(
        "(r p h f) o -> r h p (f o)", r=ROWS, p=P, h=HALVES
    )

    with tile.TileContext(nc) as tc:
        with (
            tc.tile_pool(name="xpool", bufs=10) as xpool,
            tc.tile_pool(name="small", bufs=1) as small,
        ):
            cbias = small.tile([P, 1], _f32)
            nc.vector.memset(cbias[:], C_SHIFT)

            # res[:, c] = per-partition sum of exp(chunk c + C_SHIFT).
            res = small.tile([P, N_CHUNKS], _f32)

            # All chunk loads go on the SP HWDGE ring IN ORDER: a single
            # queue delivers completions in exactly the order ACT consumes
            # them (a second ring drains its own backlog serially, so its
            # k-th transfer can complete far later than aggregate bandwidth
            # suggests, stalling the exp stream mid-kernel). One queue
            # saturates all 16 DMA engines (~425 GB/s measured).
            x_tiles = {}

            def issue_dma(c):
                xt = xpool.tile([P, FH], _f32, tag="x")
                cr, ch = divmod(c, HALVES)
                nc.sync.dma_start(out=xt[:], in_=y_chunks[cr, ch])
                x_tiles[c] = xt

            prefill = 10
            for c in range(prefill):
                issue_dma(c)
            for c in range(N_CHUNKS):
                xt = x_tiles.pop(c)
                nc.scalar.activation(
                    out=xt[:], in_=xt[:], func=_EXP, bias=cbias[:, 0:1],
                    scale=1.0, accum_out=res[:, c : c + 1],
                )
                if c + prefill < N_CHUNKS:
                    issue_dma(c + prefill)

            nc.sync.dma_start(out=out.ap(), in_=res[:])

    nc.compile()
    return nc


def make_in_maps(y_hat: np.ndarray):
    """Shard y_hat across cores (8 batch rows each)."""
    y_hat = np.ascontiguousarray(y_hat, dtype=np.float32)
    return [
        {"y": y_hat[c * ROWS : (c + 1) * ROWS].reshape(N_PER_CORE, 1)}
        for c in range(N_CORES)
    ]


def kernel(y_hat: np.ndarray, coords: np.ndarray) -> np.ndarray:
    global _compiled_nc, LAST_RESULTS
    y_hat = np.ascontiguousarray(y_hat, dtype=np.float32)
    coords = np.asarray(coords, dtype=np.float32)
    if _compiled_nc is None:
        _compiled_nc = build_nc()
    res = run_bass_kernel_spmd(
        _compiled_nc, make_in_maps(y_hat), core_ids=list(range(N_CORES))
    )
    LAST_RESULTS = res

    # Host-side finish in float64. Picked logits are pure indexing; matching
    # jnp.round (round-half-to-even) is np.round's semantics, and coords*128
    # is exact in f32 (power-of-two scale).
    xi = np.round(coords[:, :, 0] * np.float32(G)).astype(np.int64)  # (B, T)
    yi = np.round(coords[:, :, 1] * np.float32(G)).astype(np.int64)  # (B, T)
    bi = np.arange(B, dtype=np.int64)[:, None]
    ti = np.arange(T, dtype=np.int64)[None, :]
    picksum = y_hat[bi, ti, xi, yi].astype(np.float64).sum()

    lse_sum = 0.0
    for c, r in enumerate(res.results):
        o = np.asarray(r["out"]).astype(np.float64)      # [P, N_CHUNKS]
        s_rows = o.sum(axis=0).reshape(ROWS, HALVES).sum(axis=1)
        lse_sum += (np.log(s_rows) - C_SHIFT).sum()
    return np.array(np.float32((T * lse_sum - picksum) / B))


# revision 16
# speedup vs baseline: 1.0505x; 1.0099x over previous
"""Trainium2 Bass kernel for the CrossEntropyMap loss.

Math (per batch row b of y_hat[B=64, T=64, G=128, G]):
    lse_b  = logsumexp(y_hat[b].reshape(-1))            # over T*G*G = 1M classes
    pick_b = sum_t y_hat[b, t, xi[b,t], yi[b,t]]        # xi/yi = round(coords*G)
    loss   = mean_b(T * lse_b - pick_b)

Sharding: data-parallel over batch, 8 rows per NeuronCore (32 MiB/core).
The device's only job is the bandwidth-bound part: stream the 32 MiB shard
from HBM and accumulate per-partition sums of exp(x + C_SHIFT) on the ACT
engine (accum_out). Any constant shift is mathematically exact for logsumexp
(it only scales the partial sums); C_SHIFT=-16 keeps exp in range for |x| up
to ~100.

Everything else runs on the host in float64: the 4096 picked logits are pure
indexing into y_hat (already resident in host memory), and the final
cross-partition sums + ln + mean are O(B) work. Shipping only the raw
[128, 11] accumulator tile per core keeps the device critical path to
"last chunk load -> one short exp -> one 5.5 KB store" and minimizes the
instruction/semaphore count (the profiled end-of-kernel semaphore teardown
scales with it; the previous on-device reduction paid ~9 us there).

Chunking: 16 uniform half-row [128, 4096] transfers, even chunks on the SP
HWDGE ring and odd chunks on the DVE ring, strictly alternating. Queues
drain their backlogs serially, so completion ORDER must match consumption
order: a strict 50/50 interleave makes each ring's j-th transfer complete
at the same wall time as global chunk 2j/2j+1 on a single queue (the 16
shared DMA engines round-robin both queues at ~425 GB/s aggregate), while
two rings double the descriptor-generation rate at the start (a single
ring ramps ~5 us slower). An UNEVEN split stalls ACT mid-kernel (measured
+10 us). Only SP and ACT have HWDGE rings; the ACT queue has ~0.9 us of
slack per chunk for its dispatch instructions. Exp output (discarded) goes to a small
bf16 scratch: writing exp in place over the f32 input tile slowed ACT ~20%
(same-address read+write conflict), and bf16 halves the write bandwidth.
Chunk-size tapering buys nothing: the ~460 ns fixed cost per extra
activation + accumulator-readout cancels the shorter tail exp (simulated).
"""

import sys

import numpy as np

try:
    import concourse.bacc as bacc
except ImportError:  # pragma: no cover - fallback for bare environments
    sys.path.insert(0, "/opt/trn_rl_repo")
    import concourse.bacc as bacc

import concourse.tile as tile
from concourse import mybir
from concourse.bass_utils import run_bass_kernel_spmd

B, T, G = 64, 64, 128
N_CORES = 8
ROWS = B // N_CORES            # 8 batch rows per core
ROW_ELEMS = T * G * G          # 1_048_576 classes per row
P = 128
F = ROW_ELEMS // P             # 8192 elements per partition per row
N_PER_CORE = ROWS * ROW_ELEMS  # 8_388_608 elements per core shard
C_SHIFT = -16.0                # constant exp bias (exact for logsumexp)

HALVES = 2                     # chunks per row
FH = F // HALVES               # 4096 cols per chunk (2 MiB)
N_CHUNKS = ROWS * HALVES       # 16 chunk columns in the output

_f32 = mybir.dt.float32
_bf16 = mybir.dt.bfloat16
_EXP = mybir.ActivationFunctionType.Exp

_compiled_nc = None

# Test hook: BassKernelResults of the last run.
LAST_RESULTS = None


def build_nc():
    nc = bacc.Bacc("TRN2", target_bir_lowering=False, debug=False)
    y = nc.dram_tensor("y", [N_PER_CORE, 1], _f32, kind="ExternalInput")
    out = nc.dram_tensor("out", [P, N_CHUNKS], _f32, kind="ExternalOutput")

    # [ROWS, HALVES, 128, 4096] chunk view: partition p of chunk (r, h) holds
    # elements [r*1M + p*8192 + h*4096, +4096) — contiguous per partition.
    y_chunks = y.ap().rearrange(
        "(r p h f) o -> r h p (f o)", r=ROWS, p=P, h=HALVES
    )

    with tile.TileContext(nc) as tc:
        with (
            tc.tile_pool(name="xpool", bufs=10) as xpool,
            tc.tile_pool(name="escratch", bufs=2) as escratch,
            tc.tile_pool(name="small", bufs=1) as small,
        ):
            cbias = small.tile([P, 1], _f32)
            nc.vector.memset(cbias[:], C_SHIFT)

            # res[:, c] = per-partition sum of exp(chunk c + C_SHIFT).
            res = small.tile([P, N_CHUNKS], _f32)

            x_tiles = {}

            def issue_dma(c):
                xt = xpool.tile([P, FH], _f32, tag="x")
                cr, ch = divmod(c, HALVES)
                eng = nc.sync if c % 2 == 0 else nc.scalar
                eng.dma_start(out=xt[:], in_=y_chunks[cr, ch])
                x_tiles[c] = xt

            prefill = 10
            for c in range(prefill):
                issue_dma(c)
            for c in range(N_CHUNKS):
                xt = x_tiles.pop(c)
                et = escratch.tile([P, FH], _bf16, tag="e")
                nc.scalar.activation(
                    out=et[:], in_=xt[:], func=_EXP, bias=cbias[:, 0:1],
                    scale=1.0, accum_out=res[:, c : c + 1],
                )
                if c + prefill < N_CHUNKS:
                    issue_dma(c + prefill)

            nc.sync.dma_start(out=out.ap(), in_=res[:])

    nc.compile()
    return nc


def make_in_maps(y_hat: np.ndarray):
    """Shard y_hat across cores (8 batch rows each)."""
    y_hat = np.ascontiguousarray(y_hat, dtype=np.float32)
    return [
        {"y": y_hat[c * ROWS : (c + 1) * ROWS].reshape(N_PER_CORE, 1)}
        for c in range(N_CORES)
    ]


def kernel(y_hat: np.ndarray, coords: np.ndarray) -> np.ndarray:
    global _compiled_nc, LAST_RESULTS
    y_hat = np.ascontiguousarray(y_hat, dtype=np.float32)
    coords = np.asarray(coords, dtype=np.float32)
    if _compiled_nc is None:
        _compiled_nc = build_nc()
    res = run_bass_kernel_spmd(
        _compiled_nc, make_in_maps(y_hat), core_ids=list(range(N_CORES))
    )
    LAST_RESULTS = res

    # Host-side finish in float64. Picked logits are pure indexing; matching
    # jnp.round (round-half-to-even) is np.round's semantics, and coords*128
    # is exact in f32 (power-of-two scale).
    xi = np.round(coords[:, :, 0] * np.float32(G)).astype(np.int64)  # (B, T)
    yi = np.round(coords[:, :, 1] * np.float32(G)).astype(np.int64)  # (B, T)
    bi = np.arange(B, dtype=np.int64)[:, None]
    ti = np.arange(T, dtype=np.int64)[None, :]
    picksum = y_hat[bi, ti, xi, yi].astype(np.float64).sum()

    lse_sum = 0.0
    for c, r in enumerate(res.results):
        o = np.asarray(r["out"]).astype(np.float64)      # [P, N_CHUNKS]
        s_rows = o.sum(axis=0).reshape(ROWS, HALVES).sum(axis=1)
        lse_sum += (np.log(s_rows) - C_SHIFT).sum()
    return np.array(np.float32((T * lse_sum - picksum) / B))
